# revision 4
# baseline (speedup 1.0000x reference)
"""Trainium2 Bass kernel for BatteryMoE flatten intra-cycle MoE layer.

Problem (hardcoded shapes): B=128, L=128, C=3, T=512 (F=C*T=1536), E=16
experts, G=2 general experts, D=768.

    x = cycle_curve_data.reshape(B, L, F)
    g = renormalized masked softmax gates                      [B, E]
    out[b] = bf16(x[b] @ (sum_e g[b,e] W_e).T + b_eff[b]) + (x[b] @ (sum_g Wg).T + sum_g bg)

Device algorithm: fold the summed general expert in as a 17th expert with
gate 1.0, so out[b] = x[b] @ W_tot[b].T + b_tot[b] with
W_tot[b] = sum_{e<17} g_aug[b,e] W_aug[e].  (The reference's intermediate
bf16 rounding of the expert term is not replicated; its effect is ~1e-3
relative, same order as fp16 input rounding used here.)

Sharding: 8 cores = 4 batch-shards (32 samples) x 2 D-shards (384 outs).
Per core, both stages run on the TensorEngine in fp16:
  1) combine: for each output row-block, W_tot chunks are built by K=17
     matmuls (lhsT = W_aug[:, d, f_blk] [17,128], rhs = gT [17,32]) that
     produce W_tot directly in the transposed [f, d, b] layout,
  2) main: out[b][l, d_blk] = sum_t xT[b,f_t,:].T @ W3[f_t, d_blk, b],
     plus a K=1 ones-row matmul adding the per-sample bias.
x is transposed/packed to fp16 on the host (input formatting), outputs are
f32 and concatenated on the host.
"""

import numpy as np

import concourse.bass as bass
import concourse.tile as tile
from concourse import bacc, mybir
from concourse.bass_utils import run_bass_kernel_spmd

B, L, F, D, E = 128, 128, 1536, 768, 16
EAUG = E + 1
N_CORES = 8
B_SHARDS, D_SHARDS = 4, 2
B_LOC = B // B_SHARDS          # 32
D_LOC = D // D_SHARDS          # 384
D_BLK = 128                    # output-column block
N_BLK = D_LOC // D_BLK         # 3
N_T = F // 128                 # 12 contraction tiles
SLAB_D = 8                     # W d-rows per streamed slab
N_SLAB = D_BLK // SLAB_D       # 16 slabs per block
FP16 = mybir.dt.float16
F32 = mybir.dt.float32

_PROGRAM = None


def _build_program():
    nc = bacc.Bacc("TRN2", target_bir_lowering=False, debug=False,
                   num_devices=N_CORES)

    xt_d = nc.dram_tensor("xt", [B_LOC // 2, 128, N_T * 256], FP16,
                          kind="ExternalInput")
    w_d = nc.dram_tensor("w", [EAUG, D_LOC, F], FP16, kind="ExternalInput")
    gt_d = nc.dram_tensor("gt", [EAUG, B_LOC], FP16, kind="ExternalInput")
    bias_d = nc.dram_tensor("bias", [1, B_LOC * D_LOC], FP16, kind="ExternalInput")
    out_d = nc.dram_tensor("out", [B_LOC, L, D_LOC], F32, kind="ExternalOutput")

    with tile.TileContext(nc) as tc:
        with (
            tc.tile_pool(name="const", bufs=1) as cpool,
            tc.tile_pool(name="w3p", bufs=1) as w3pool,
            tc.tile_pool(name="wslab", bufs=2) as wpool,
            tc.tile_pool(name="xtp", bufs=2) as xtpool,
            tc.tile_pool(name="outp", bufs=4) as opool,
            tc.tile_pool(name="psc", bufs=3, space="PSUM") as psumc,
            tc.tile_pool(name="pso", bufs=4, space="PSUM") as psumo,
        ):
            gt = cpool.tile([EAUG, B_LOC], FP16)
            nc.sync.dma_start(gt[:], gt_d.ap())
            bias = cpool.tile([1, B_LOC * D_LOC], FP16)
            nc.sync.dma_start(bias[:], bias_d.ap())
            ones = cpool.tile([1, 128], FP16)
            nc.vector.memset(ones[:], 1.0)

            # Persistent W_tot chunk store: [f_part, t, d_in_blk, b] fp16.
            w3 = w3pool.tile([128, N_T, D_BLK, B_LOC], FP16)

            ncopy = 0
            for blk in range(N_BLK):
                # --- combine: build W3 for this d block ---
                for sl in range(N_SLAB):
                    d0 = blk * D_BLK + sl * SLAB_D
                    slab = wpool.tile([EAUG, SLAB_D, F], FP16)
                    nc.sync.dma_start(slab[:], w_d[:, d0:d0 + SLAB_D, :])
                    for t in range(N_T):
                        pc = psumc.tile([128, SLAB_D * B_LOC], F32)
                        for d8 in range(SLAB_D):
                            nc.tensor.matmul(
                                pc[:, d8 * B_LOC:(d8 + 1) * B_LOC],
                                slab[:, d8, t * 128:(t + 1) * 128],
                                gt[:],
                                start=True, stop=True,
                            )
                        dst = w3[:, t, sl * SLAB_D:(sl + 1) * SLAB_D, :]
                        if ncopy % 3 == 2:
                            nc.scalar.copy(dst, pc[:])
                        else:
                            nc.vector.tensor_copy(dst, pc[:])
                        ncopy += 1

                # --- main: per-sample matmuls for this d block ---
                for pair in range(B_LOC // 2):
                    xt_t = xtpool.tile([128, N_T, 256], FP16)
                    nc.sync.dma_start(xt_t[:], xt_d[pair])
                    for j in range(2):
                        b = pair * 2 + j
                        po = psumo.tile([128, D_BLK], F32)
                        for t in range(N_T):
                            nc.tensor.matmul(
                                po[:],
                                xt_t[:, t, j * 128:(j + 1) * 128],
                                w3[:, t, :, b],
                                start=(t == 0), stop=False,
                            )
                        nc.tensor.matmul(
                            po[:],
                            ones[:],
                            bias[0:1, b * D_LOC + blk * D_BLK:
                                 b * D_LOC + (blk + 1) * D_BLK],
                            start=False, stop=True,
                        )
                        ot = opool.tile([128, D_BLK], F32)
                        if ncopy % 3 == 2:
                            nc.scalar.copy(ot[:], po[:])
                        else:
                            nc.vector.tensor_copy(ot[:], po[:])
                        ncopy += 1
                        nc.sync.dma_start(
                            out_d[b, :, blk * D_BLK:(blk + 1) * D_BLK], ot[:]
                        )

    nc.compile()
    return nc


def kernel(cycle_curve_data, logits, moe_masks, expert_W, expert_b,
           gen_W, gen_b):
    global _PROGRAM
    x = np.asarray(cycle_curve_data, dtype=np.float32).reshape(B, L, F)
    logits = np.asarray(logits, dtype=np.float32)

    # Gates: masked renormalized softmax, f32 exactly as the reference.
    m = (np.asarray(moe_masks) == 1).astype(np.float32)
    ex = np.exp(logits - logits.max(axis=1, keepdims=True))
    sm = ex / ex.sum(axis=1, keepdims=True)
    g = sm * m
    g = g / (g.sum(axis=1, keepdims=True) + 1e-9)

    g_aug = np.concatenate([g, np.ones((B, 1), np.float32)], axis=1)
    W_aug = np.concatenate(
        [np.asarray(expert_W, np.float32),
         np.asarray(gen_W, np.float32).sum(axis=0, keepdims=True)],
        axis=0).astype(np.float16)                                  # [17, D, F]
    b_tot = (g @ np.asarray(expert_b, np.float32)
             + np.asarray(gen_b, np.float32).sum(axis=0)).astype(np.float16)

    # xT packed per core: [pair, partition(f%?), t, j, l] -> [16, 128, 12*256]
    xt = np.ascontiguousarray(x.transpose(0, 2, 1)).astype(np.float16)  # [B,F,L]

    in_maps = []
    for core in range(N_CORES):
        bi, dj = divmod(core, D_SHARDS)
        xs = xt[bi * B_LOC:(bi + 1) * B_LOC]                        # [32,1536,128]
        xs = xs.reshape(B_LOC // 2, 2, N_T, 128, L)                  # pair,j,t,p,l
        xs = np.ascontiguousarray(xs.transpose(0, 3, 2, 1, 4))       # pair,p,t,j,l
        in_maps.append({
            "xt": xs.reshape(B_LOC // 2, 128, N_T * 256),
            "w": np.ascontiguousarray(W_aug[:, dj * D_LOC:(dj + 1) * D_LOC, :]),
            "gt": np.ascontiguousarray(
                g_aug[bi * B_LOC:(bi + 1) * B_LOC].T).astype(np.float16),
            "bias": np.ascontiguousarray(
                b_tot[bi * B_LOC:(bi + 1) * B_LOC,
                      dj * D_LOC:(dj + 1) * D_LOC]).reshape(1, B_LOC * D_LOC),
        })

    if _PROGRAM is None:
        _PROGRAM = _build_program()
    res = run_bass_kernel_spmd(_PROGRAM, in_maps, list(range(N_CORES)))

    out = np.empty((B, L, D), dtype=np.float32)
    for core in range(N_CORES):
        bi, dj = divmod(core, D_SHARDS)
        out[bi * B_LOC:(bi + 1) * B_LOC, :,
            dj * D_LOC:(dj + 1) * D_LOC] = res.results[core]["out"]
    return out
